# revision 12
# baseline (speedup 1.0000x reference)
# Bass/Trainium2 kernel for BatchOnlineNorm (online control-normalization
# with batch-sequential EMA stats + per-sample RMS layer scaling).
#
# Strategy (8 cores, H-sharded):
#  - Each core owns 8 of the 64 H-rows: x-shard [32, 512, 256] (16 MiB), kept
#    RESIDENT in SBUF.
#  - Pass 1: per-sample per-channel spatial sums S1=sum(x), S2=sum(x^2) via
#    TensorE matmuls (one-hot lhsT routes sample t to PSUM partition t),
#    accumulated over the whole pass in two PSUM banks.
#  - One 64 KiB AllReduce over the 8 cores combines partial sums.
#  - The sequential EMA recurrence has a closed form: mu_prev = L@m1 (+a^t mu0),
#    var_prev = V@e2 (+a^t var0) with small lower-triangular matrices baked in
#    as NEFF consts, so it becomes two tiny [32,32]x[32,256] matmuls plus a few
#    vectorized [32,256] DVE ops; the per-sample RMS (layer scaling) also
#    reduces to a closed form over the same stats.
#  - Pass 2: out = x * A[t,c] + B[t,c]; A,B rows are broadcast across the 128
#    spatial partitions with a K=1 outer-product matmul, applied with two DVE
#    tensor-tensor ops in place over the resident x, then DMA'd out.
import numpy as np

AFWD = 0.999
EPS = 1e-5
B, H, W, C = 32, 64, 64, 256
NCORES = 8
HPC = H // NCORES      # H-rows per core
SP = HPC * W           # spatial elements per core per sample (512)
TOT_SP = H * W         # 4096 (full spatial for the means)


def _recurrence_consts(nb, tot_sp):
    """Closed-form coefficient matrices for the EMA recurrence (float64)."""
    a = float(AFWD)
    # mu_prev[t] = a^t * mu0 + sum_{i<t} (1-a) a^(t-1-i) * m1[i]
    tri_mu = np.zeros((nb, nb), dtype=np.float64)   # lhsT: [i, t]
    # var_prev[t] = a^t * var0 + sum_{i<t} (1-a) a^(t-i) * e2[i]
    tri_v = np.zeros((nb, nb), dtype=np.float64)
    init = np.zeros((1, nb), dtype=np.float64)      # lhsT: [0, t] = a^t
    for t in range(nb):
        init[0, t] = a ** t
        for i in range(t):
            tri_mu[i, t] = (1.0 - a) * a ** (t - 1 - i)
            tri_v[i, t] = (1.0 - a) * a ** (t - i)
    return (tri_mu.astype(np.float32), tri_v.astype(np.float32),
            init.astype(np.float32))


def build_tile_body(tc, outs, ins, nb, sp, c, ncores):
    """Emit the kernel body into TileContext tc.

    ins: dict of DRAM APs {xs, gamma, beta, stream_mu, stream_var}
    outs: dict {ys}
    nb: batch size, sp: spatial per core, c: channels, ncores: replica count.
    """
    from contextlib import ExitStack
    import concourse.bass as bass
    from concourse import mybir
    f32 = mybir.dt.float32
    bf16 = mybir.dt.bfloat16
    AX = mybir.AxisListType
    OP = mybir.AluOpType
    ACT = mybir.ActivationFunctionType

    nc = tc.nc
    assert sp % 128 == 0
    S = sp // 128              # free-dim chunks of 128 spatial each
    SP2 = min(S, 2)            # chunks after pairing into N<=512 matmuls
    tot_sp = sp * ncores

    xs = ins["xs"]             # [nb, sp, c]
    gamma = ins["gamma"]       # [1, c]
    beta = ins["beta"]
    mu0_d = ins["stream_mu"]
    var0_d = ins["stream_var"]
    ys = outs["ys"]

    tri_mu_np, tri_v_np, init_np = _recurrence_consts(nb, tot_sp)
    tri_mu_d = nc.inline_tensor(tri_mu_np, name="tri_mu")
    tri_v_d = nc.inline_tensor(tri_v_np, name="tri_v")
    init_d = nc.inline_tensor(init_np, name="init_pow")
    import ml_dtypes
    oh_np = np.zeros((128, nb, nb), dtype=ml_dtypes.bfloat16)
    for t in range(nb):
        oh_np[:, t, t] = 1.0
    oh_d = nc.inline_tensor(oh_np, name="onehots")
    # row-selector for the pass-2 broadcast: rowsel[k, t, m] = (k == t)
    rowsel_np = np.zeros((nb, nb, 128), dtype=np.float32)
    for t in range(nb):
        rowsel_np[t, t, :] = 1.0
    rowsel_d = nc.inline_tensor(rowsel_np, name="rowsel")

    ctx = ExitStack()
    with ctx:
        big = ctx.enter_context(tc.tile_pool(name="big", bufs=1))
        sqp = ctx.enter_context(tc.tile_pool(name="sqp", bufs=3))
        cst = ctx.enter_context(tc.tile_pool(name="cst", bufs=1))
        mid = ctx.enter_context(tc.tile_pool(name="mid", bufs=1))
        pp_stats = ctx.enter_context(
            tc.tile_pool(name="pp_stats", bufs=1, space="PSUM"))
        pp_mid = ctx.enter_context(
            tc.tile_pool(name="pp_mid", bufs=1, space="PSUM"))
        pp_bc = ctx.enter_context(
            tc.tile_pool(name="pp_bc", bufs=3, space="PSUM"))
        dram = ctx.enter_context(
            tc.tile_pool(name="dram", bufs=1, space="DRAM"))

        # ---- constants / small loads -------------------------------------
        gamma_bc = cst.tile([nb, c], f32)
        nc.gpsimd.dma_start(out=gamma_bc, in_=bass.AP(
            tensor=gamma.tensor, offset=gamma.offset, ap=[[0, nb], [1, c]]))
        beta_bc = cst.tile([nb, c], f32)
        nc.gpsimd.dma_start(out=beta_bc, in_=bass.AP(
            tensor=beta.tensor, offset=beta.offset, ap=[[0, nb], [1, c]]))
        mu0_sb = cst.tile([1, c], f32)
        nc.gpsimd.dma_start(out=mu0_sb, in_=mu0_d)
        var0_sb = cst.tile([1, c], f32)
        nc.gpsimd.dma_start(out=var0_sb, in_=var0_d)
        tri_mu_sb = cst.tile([nb, nb], f32)
        nc.gpsimd.dma_start(out=tri_mu_sb, in_=tri_mu_d.ap())
        tri_v_sb = cst.tile([nb, nb], f32)
        nc.gpsimd.dma_start(out=tri_v_sb, in_=tri_v_d.ap())
        init_sb = cst.tile([1, nb], f32)
        nc.gpsimd.dma_start(out=init_sb, in_=init_d.ap())
        oh_sb = cst.tile([128, nb, nb], bf16)
        nc.gpsimd.dma_start(out=oh_sb, in_=oh_d.ap())
        rowsel_sb = cst.tile([nb, nb, 128], f32)
        nc.gpsimd.dma_start(out=rowsel_sb, in_=rowsel_d.ap())

        eps_col = cst.tile([nb, 1], f32)
        nc.vector.memset(eps_col, EPS)

        # sum_c beta^2 (constant part of the per-sample RMS)
        bsq = mid.tile([nb, c], f32)
        nc.vector.tensor_mul(bsq, beta_bc, beta_bc)
        betasq_sum = mid.tile([nb, 1], f32)
        nc.vector.reduce_sum(betasq_sum, bsq, axis=AX.X)

        # ---- pass 1: resident load + spatial sums ------------------------
        xr = big.tile([128, nb, S, c], f32)
        psum_s1 = pp_stats.tile([nb, SP2, c], f32)
        psum_s2 = pp_stats.tile([nb, SP2, c], f32)

        n_mm = (S + SP2 - 1) // SP2     # matmuls per sample per stat
        for t in range(nb):
            nc.sync.dma_start(
                out=xr[:, t],
                in_=xs[t].rearrange("(p s) c -> p s c", s=S))
            # bf16 casts feed the PE stat-reduction (full-rate matmuls; the
            # rounding error is damped by the (1-a)=1e-3 EMA coefficient)
            xb = sqp.tile([128, S, c], bf16, name="xb")
            nc.scalar.copy(xb, xr[:, t])
            sq = sqp.tile([128, S, c], bf16, name="sq")
            nc.vector.tensor_mul(sq, xr[:, t], xr[:, t])
            lhsT = oh_sb[:, t, :]
            for k in range(n_mm):
                s0, s1_ = k * SP2, min((k + 1) * SP2, S)
                first = (t == 0 and k == 0)
                last = (t == nb - 1 and k == n_mm - 1)
                nc.tensor.matmul(
                    psum_s1[:, 0:(s1_ - s0), :], lhsT,
                    xb[:, s0:s1_, :],
                    start=first, stop=last)
                nc.tensor.matmul(
                    psum_s2[:, 0:(s1_ - s0), :], lhsT,
                    sq[:, s0:s1_, :],
                    start=first, stop=last)

        # fold the SP2 chunks and stage for the collective: [nb, 2c]
        stats_sb = mid.tile([nb, 2 * c], f32)
        st1 = mid.tile([nb, SP2, c], f32)
        nc.scalar.copy(st1, psum_s1[:nb])
        st2 = mid.tile([nb, SP2, c], f32)
        nc.scalar.copy(st2, psum_s2[:nb])
        if SP2 == 2:
            nc.vector.tensor_add(stats_sb[:, 0:c], st1[:, 0, :], st1[:, 1, :])
            nc.vector.tensor_add(stats_sb[:, c:2 * c], st2[:, 0, :], st2[:, 1, :])
        else:
            nc.vector.tensor_copy(stats_sb[:, 0:c], st1[:, 0, :])
            nc.vector.tensor_copy(stats_sb[:, c:2 * c], st2[:, 0, :])

        # ---- cross-core AllReduce of the [nb, 2c] partial sums -----------
        if ncores > 1:
            cc_space = "Shared" if ncores > 4 else "Local"
            cc_in = dram.tile([nb, 2 * c], f32)
            cc_out = dram.tile([nb, 2 * c], f32, addr_space=cc_space)
            nc.gpsimd.dma_start(out=cc_in, in_=stats_sb)
            nc.gpsimd.collective_compute(
                "AllReduce", OP.add,
                replica_groups=[list(range(ncores))],
                ins=[cc_in.opt()], outs=[cc_out.opt()])
            stats_full = mid.tile([nb, 2 * c], f32)
            nc.gpsimd.dma_start(out=stats_full, in_=cc_out)
        else:
            stats_full = stats_sb

        # ---- closed-form recurrence --------------------------------------
        m1 = mid.tile([nb, c], f32)
        nc.vector.tensor_scalar_mul(m1, stats_full[:, 0:c], 1.0 / tot_sp)
        m2 = mid.tile([nb, c], f32)
        nc.vector.tensor_scalar_mul(m2, stats_full[:, c:2 * c], 1.0 / tot_sp)

        psum_mu = pp_mid.tile([nb, c], f32)
        nc.tensor.matmul(psum_mu, tri_mu_sb, m1, start=True, stop=False)
        nc.tensor.matmul(psum_mu, init_sb, mu0_sb, start=False, stop=True)

        d1 = mid.tile([nb, c], f32)      # m1 - mu_prev
        nc.vector.tensor_sub(d1, m1, psum_mu)
        tmp = mid.tile([nb, c], f32)     # 2*m1 - mu_prev
        nc.vector.tensor_add(tmp, m1, d1)
        t2 = mid.tile([nb, c], f32)      # mu_prev * (2*m1 - mu_prev)
        nc.vector.tensor_mul(t2, psum_mu, tmp)
        e2 = mid.tile([nb, c], f32)      # E[(x - mu_prev)^2]
        nc.vector.tensor_sub(e2, m2, t2)

        psum_var = pp_mid.tile([nb, c], f32)
        nc.tensor.matmul(psum_var, tri_v_sb, e2, start=True, stop=False)
        nc.tensor.matmul(psum_var, init_sb, var0_sb, start=False, stop=True)

        sv = mid.tile([nb, c], f32)      # sqrt(var_prev + eps)
        nc.scalar.activation(sv, psum_var, ACT.Sqrt, bias=eps_col, scale=1.0)
        iv = mid.tile([nb, c], f32)      # 1/sqrt(var_prev + eps)
        nc.vector.reciprocal(iv, sv)

        a0 = mid.tile([nb, c], f32)      # gamma * iv
        nc.vector.tensor_mul(a0, gamma_bc, iv)
        am = mid.tile([nb, c], f32)      # a0 * mu_prev
        nc.vector.tensor_mul(am, a0, psum_mu)
        c0 = mid.tile([nb, c], f32)      # beta - a0*mu_prev
        nc.vector.tensor_sub(c0, beta_bc, am)

        # per-sample RMS: ms = (1/c) * sum_c [a0^2 e2 + 2 a0 beta d1 + beta^2]
        u = mid.tile([nb, c], f32)
        nc.vector.tensor_mul(u, a0, e2)
        v = mid.tile([nb, c], f32)
        nc.vector.tensor_mul(v, beta_bc, d1)
        w = mid.tile([nb, c], f32)
        nc.vector.scalar_tensor_tensor(w, v, 2.0, u, op0=OP.mult, op1=OP.add)
        term = mid.tile([nb, c], f32)
        nc.vector.tensor_mul(term, a0, w)
        ms = mid.tile([nb, 1], f32)
        nc.vector.reduce_sum(ms, term, axis=AX.X)
        nc.vector.tensor_add(ms, ms, betasq_sum)
        rs = mid.tile([nb, 1], f32)
        nc.scalar.activation(rs, ms, ACT.Sqrt, bias=eps_col, scale=1.0 / c)
        r = mid.tile([nb, 1], f32)
        nc.vector.reciprocal(r, rs)

        ab = mid.tile([nb, 2 * c], f32)  # [A | B] rows
        nc.vector.tensor_scalar_mul(ab[:, 0:c], a0, r)
        nc.vector.tensor_scalar_mul(ab[:, c:2 * c], c0, r)

        # ---- pass 2: out = x*A + B, in place over the resident shard -----
        for t in range(nb):
            ab_ps = pp_bc.tile([128, 2 * c], f32, name="ab_ps")
            nc.tensor.matmul(ab_ps, rowsel_sb[:, t, :], ab,
                             start=True, stop=True)
            a_view = ab_ps[:, 0:c].unsqueeze(1).to_broadcast((128, S, c))
            b_view = ab_ps[:, c:2 * c].unsqueeze(1).to_broadcast((128, S, c))
            nc.vector.tensor_mul(xr[:, t], xr[:, t], a_view)
            nc.vector.tensor_add(xr[:, t], xr[:, t], b_view)
            nc.sync.dma_start(
                out=ys[t].rearrange("(p s) c -> p s c", s=S),
                in_=xr[:, t])


def build_nc(nb=B, sp=SP, c=C, ncores=NCORES):
    import concourse.bacc as bacc
    import concourse.tile as tile
    from concourse import mybir
    f32 = mybir.dt.float32

    nc = bacc.Bacc("TRN2", target_bir_lowering=False, debug=False,
                   num_devices=ncores)
    xs = nc.dram_tensor("xs", [nb, sp, c], f32, kind="ExternalInput")
    gamma = nc.dram_tensor("gamma", [1, c], f32, kind="ExternalInput")
    beta = nc.dram_tensor("beta", [1, c], f32, kind="ExternalInput")
    mu0 = nc.dram_tensor("stream_mu", [1, c], f32, kind="ExternalInput")
    var0 = nc.dram_tensor("stream_var", [1, c], f32, kind="ExternalInput")
    ys = nc.dram_tensor("ys", [nb, sp, c], f32, kind="ExternalOutput")

    ins = {"xs": xs.ap(), "gamma": gamma.ap(), "beta": beta.ap(),
           "stream_mu": mu0.ap(), "stream_var": var0.ap()}
    outs = {"ys": ys.ap()}
    with tile.TileContext(nc) as tc:
        build_tile_body(tc, outs, ins, nb, sp, c, ncores)
    nc.compile()
    return nc


_cached_nc = None
LAST_RESULTS = None  # BassKernelResults of the most recent kernel() call


def kernel(**inputs):
    global _cached_nc, LAST_RESULTS
    from concourse.bass_utils import run_bass_kernel_spmd

    x = np.ascontiguousarray(np.asarray(inputs["x"], dtype=np.float32))
    gamma = np.asarray(inputs["gamma"], dtype=np.float32).reshape(1, C)
    beta = np.asarray(inputs["beta"], dtype=np.float32).reshape(1, C)
    mu0 = np.asarray(inputs["stream_mu"], dtype=np.float32).reshape(1, C)
    var0 = np.asarray(inputs["stream_var"], dtype=np.float32).reshape(1, C)

    if _cached_nc is None:
        _cached_nc = build_nc()
    nc = _cached_nc

    in_maps = []
    for k in range(NCORES):
        xs_k = np.ascontiguousarray(
            x[:, k * HPC:(k + 1) * HPC].reshape(B, SP, C))
        in_maps.append({"xs": xs_k, "gamma": gamma, "beta": beta,
                        "stream_mu": mu0, "stream_var": var0})

    import os
    trace = bool(os.environ.get("KERNEL_TRACE"))
    res = run_bass_kernel_spmd(nc, in_maps, core_ids=list(range(NCORES)),
                               trace=trace)
    LAST_RESULTS = res

    y = np.empty((B, H, W, C), dtype=np.float32)
    for k in range(NCORES):
        y[:, k * HPC:(k + 1) * HPC] = res.results[k]["ys"].reshape(
            B, HPC, W, C)
    return y
